# revision 1
# baseline (speedup 1.0000x reference)
"""Trainium2 Bass kernel for nn_ClusterisedSelfAttentionNotLearnable.

Computes, for each point n (N=200000, data-parallel over 8 NeuronCores):
    enc    = posenc(X[n], 6 freqs)                      # [72]
    rgbc   = (enc @ L.T).reshape(256, 3)                # [256, 3]
    attn   = softmax(X[n, :3] @ cent.T)                 # [256]
    out[n] = attn @ rgbc                                # [3]

Device pipeline per 512-point chunk (features-on-partitions, points on
the free axis):
  PE:  angle expansion (turns units, hi/lo fp16 split of x/2pi, exact
       2^f weights), cluster scores, G = Lhat.T @ escore, and the final
       block-ones reduction.
  DVE: range reduction r = (ang+32) mod 1, and P = enc * G.
  ACT: sin2pi (shares the exp_and_friends table set with exp -> no ACT
       table switches) and exp.
The softmax division happens on the host from the returned [4, Npts]
(3 numerators + denominator) slab.
"""

import os as _os
import sys

sys.path.insert(0, "/opt/trn_rl_repo")

import ml_dtypes
import numpy as np

import concourse.bass as bass
import concourse.tile as tile
from concourse import mybir
from concourse.bass_utils import run_bass_kernel_spmd
from concourse.tile import TileContext, ScopedClock

# ---------------------------------------------------------------- constants
N = 200000
D = 6
C = 256
NFREQ = 6
NCORES = 8
NPC = N // NCORES           # 25000 points per core
NF = 512                    # points per chunk
CH = 50                     # chunks per core (even: rgbd escapes in pairs)
NPAD = CH * NF              # 25600 padded points per core

F16 = mybir.dt.float16
BF16 = mybir.dt.bfloat16
F32 = mybir.dt.float32
NP_BF16 = ml_dtypes.bfloat16

_TWO_PI = 2.0 * np.pi

# ------------------------------------------------- harness compatibility patches


def _patch_tile_drain():
    """This walrus build rejects >2 sync waits on one instruction; spread the
    TileContext tail-drain waits across single-wait NOPs."""

    def _drain_and_barrier(self, tick_clock, wait_clock):
        nc = self.nc
        carrier = nc.sync.nop(nofuse=True)
        wait_clock.add_sem_waits(
            carrier.ins, ScopedClock({None: tick_clock.global_clock})
        )
        si = carrier.ins.sync_info
        waits = list(si.on_wait or []) if si is not None else []
        if len(waits) > 1:
            si.on_wait = waits[:1]
            for w in waits[1:]:
                extra = nc.sync.nop(nofuse=True)
                if extra.ins.sync_info is None:
                    extra.ins.sync_info = mybir.SyncInfo(on_wait=[w], on_update=[])
                else:
                    extra.ins.sync_info.on_wait = [w]
        nc.sync.drain()
        nc.all_engine_barrier()
        assert self.sems is not None
        popped = nc._tile_sem_poison_stack.pop()
        assert popped is self._sem_poison
        nc.clear_and_free_semaphores(list(self.sems.allocated().values()))
        nc.all_engine_barrier()

    TileContext._drain_and_barrier = _drain_and_barrier


def _split_excess_waits(nc, max_waits=1):
    """This walrus build accepts at most one sync wait per data instruction.
    Move excess waits onto injected same-engine NoOps placed directly before
    the over-subscribed instruction (waiting earlier on the same engine is
    semantically identical)."""
    ctr = 0
    for f in nc.m.functions:
        for bb in f.blocks:
            il = bb.instructions
            if not any(
                i.sync_info is not None
                and i.sync_info.on_wait
                and len(i.sync_info.on_wait) > max_waits
                for i in il
            ):
                continue
            new = []
            for inst in il:
                si = inst.sync_info
                waits = list(si.on_wait) if (si is not None and si.on_wait) else []
                if len(waits) > max_waits:
                    for w in waits[: len(waits) - max_waits]:
                        nop = mybir.InstNoOp(name=f"wsplit_nop_{ctr}", ins=[], outs=[])
                        ctr += 1
                        nop.engine = inst.engine
                        nop.sync_info = mybir.SyncInfo(on_wait=[w], on_update=[])
                        new.append(nop)
                    si.on_wait = waits[len(waits) - max_waits:]
                new.append(inst)
            bb.instructions = new


def _patch_sin2pi():
    """sin2pi (ACT func id 99) is not in the concourse enum but lives in the
    exp_and_friends table set. Emit Arctan as a marker and rewrite the
    serialized BIR."""
    if getattr(bass.Bass, "_sin2pi_patched", False):
        return
    orig = bass.Bass.to_json_bytes

    def to_json_bytes(self, *a, **k):
        return orig(self, *a, **k).replace(b'"Arctan"', b'"Sin2pi"')

    bass.Bass.to_json_bytes = to_json_bytes
    bass.Bass._sin2pi_patched = True


_patch_tile_drain()
_patch_sin2pi()

SIN2PI = mybir.ActivationFunctionType.Arctan  # rewritten to Sin2pi in the BIR

# ---------------------------------------------------------------- row maps
# Feature row k in [0, 108): k = dout*36 + din*6 + f.
_K = np.arange(108)
_DOUT = _K // 36
_DIN = (_K % 36) // 6
_F = _K % 6

# x13 row layout: [thi0..2, tlo0..2, thi3..5, tlo3..5, ones]


def _j_hi(d):
    return np.where(d < 3, d, d + 3)


def _j_lo(d):
    return np.where(d < 3, d + 3, d + 6)


def _build_static_arrays(linear_mappings, centroids):
    L = np.asarray(linear_mappings, dtype=np.float32)       # [768, 72]
    cent = np.asarray(centroids, dtype=np.float32)          # [256, 3]

    # Angle-expansion weights (in turns): ang_k = 2^f * (thi + tlo)[din]
    wea = np.zeros((13, 108), dtype=np.float16)
    web = np.zeros((13, 109), dtype=np.float16)
    pw = (2.0 ** _F).astype(np.float16)
    wea[_j_hi(_DIN), _K] = pw
    wea[_j_lo(_DIN), _K] = pw
    web[:, :108] = wea[:, :108]
    # ones-row: the .25 on the cos tile is the quarter-turn shift
    # sin2pi(t+.25) = cos(2 pi t).
    web[12, :] = 0.25

    # Scores: score_c = sum_d x_d * cent[c, d];  x = 2 pi t
    cent2pi = (cent * _TWO_PI).astype(np.float16)           # [256, 3]
    c6 = np.zeros((6, 256), dtype=np.float16)
    for j in range(6):
        c6[j, :] = cent2pi[:, j % 3]

    # Lhat: G_A[k] pairs with sin rows, G_B with cos rows, col 108 = denom.
    lhata = np.zeros((256, 108), dtype=np.float32)
    lhatb = np.zeros((256, 109), dtype=np.float32)
    ecol_sin = _DIN * 12 + _F
    ecol_cos = _DIN * 12 + 6 + _F
    for k in range(108):
        lhata[:, k] = L[3 * np.arange(C) + _DOUT[k], ecol_sin[k]]
        lhatb[:, k] = L[3 * np.arange(C) + _DOUT[k], ecol_cos[k]]
    lhatb[:, 108] = 1.0

    lp = np.zeros((128, 434), dtype=NP_BF16)
    lp[:, 0:108] = lhata[0:128]
    lp[:, 108:216] = lhata[128:256]
    lp[:, 216:325] = lhatb[0:128]
    lp[:, 325:434] = lhatb[128:256]

    rp = np.zeros((109, 8), dtype=NP_BF16)
    rp[_K, _DOUT] = 1.0            # RA: cols 0..3
    rp[_K, 4 + _DOUT] = 1.0        # RB
    rp[108, 7] = 1.0               # denominator -> output row 3

    negi = (-np.eye(109, dtype=np.float16))

    return wea, web, c6, lp, rp, negi


def _build_x13(X):
    """Per-core [13, NPAD] fp16 slabs: hi/lo split of x/(2 pi) + ones row."""
    t = (np.asarray(X, dtype=np.float64) / _TWO_PI)          # [N, 6]
    thi = t.astype(np.float16)
    tlo = (t - thi.astype(np.float64)).astype(np.float16)
    x13 = np.zeros((NCORES, 13, NPAD), dtype=np.float16)
    for c in range(NCORES):
        seg = slice(c * NPC, (c + 1) * NPC)
        for d in range(6):
            x13[c, int(_j_hi(np.array(d))), :NPC] = thi[seg, d]
            x13[c, int(_j_lo(np.array(d))), :NPC] = tlo[seg, d]
        x13[c, 12, :] = 1.0
    return x13


def _build_program():
    nc = bass.Bass()
    x13_h = nc.dram_tensor("x13", [13, NPAD], F16, kind="ExternalInput")
    wea_h = nc.dram_tensor("wea", [13, 108], F16, kind="ExternalInput")
    web_h = nc.dram_tensor("web", [13, 109], F16, kind="ExternalInput")
    c6_h = nc.dram_tensor("c6", [6, 256], F16, kind="ExternalInput")
    lp_h = nc.dram_tensor("lp", [128, 434], BF16, kind="ExternalInput")
    rp_h = nc.dram_tensor("rp", [109, 8], BF16, kind="ExternalInput")
    negi_h = nc.dram_tensor("negi", [109, 109], F16, kind="ExternalInput")
    o4_h = nc.dram_tensor("o4", [4, NPAD], F32, kind="ExternalOutput")

    MAGIC = np.float32(1.5 * 2.0 ** 23)  # fp32 round-to-int constant

    EXP = mybir.ActivationFunctionType.Exp

    with TileContext(nc) as tc:
        with (
            tc.tile_pool(name="statics", bufs=1) as statics,
            tc.tile_pool(name="xin", bufs=int(_os.environ.get("KB_X", 3))) as xpool,
            tc.tile_pool(name="rmod", bufs=int(_os.environ.get("KB_M", 2))) as modpool,
            tc.tile_pool(name="enc", bufs=int(_os.environ.get("KB_E", 2))) as encpool,
            tc.tile_pool(name="esc", bufs=int(_os.environ.get("KB_S", 2))) as escpool,
            tc.tile_pool(name="pprod", bufs=int(_os.environ.get("KB_P", 2))) as ppool,
            tc.tile_pool(name="rgbs", bufs=int(_os.environ.get("KB_R", 2))) as rspool,
            tc.tile_pool(name="ang", bufs=int(_os.environ.get("KB_ANG", 1)), space="PSUM") as angpool,
            tc.tile_pool(name="sc", bufs=int(_os.environ.get("KB_SC", 1)), space="PSUM") as scpool,
            tc.tile_pool(name="g", bufs=int(_os.environ.get("KB_G", 2)), space="PSUM") as gpool,
        ):
            wea_t = statics.tile([13, 108], F16)
            web_t = statics.tile([13, 109], F16)
            c6_t = statics.tile([6, 256], F16)
            lp_t = statics.tile([128, 434], BF16)
            rp_t = statics.tile([109, 8], BF16)
            negi_t = statics.tile([109, 109], F16)
            nc.sync.dma_start(out=wea_t[:], in_=wea_h[:])
            nc.sync.dma_start(out=web_t[:], in_=web_h[:])
            nc.sync.dma_start(out=c6_t[:], in_=c6_h[:])
            nc.sync.dma_start(out=lp_t[:], in_=lp_h[:])
            nc.sync.dma_start(out=rp_t[:], in_=rp_h[:])
            nc.sync.dma_start(out=negi_t[:], in_=negi_h[:])

            for i in range(CH):
                    s = i * NF
                    xt = xpool.tile([13, NF], F16)
                    nc.sync.dma_start(out=xt[:], in_=x13_h[:, s:s + NF])

                    ang = angpool.tile([109, 2 * NF], F32)
                    sc = scpool.tile([128, 2 * NF], F32)
                    nc.tensor.matmul(
                        ang[0:108, 0:NF], wea_t[:], xt[:], start=True, stop=False
                    )
                    nc.tensor.matmul(
                        ang[0:109, NF:2 * NF], web_t[:], xt[:],
                        start=True, stop=False,
                    )
                    nc.tensor.matmul(
                        sc[:, 0:NF], c6_t[:, 0:128], xt[0:6, :],
                        start=True, stop=True,
                    )
                    nc.tensor.matmul(
                        sc[:, NF:2 * NF], c6_t[:, 128:256], xt[0:6, :],
                        start=True, stop=True,
                    )

                    # k = rint(ang) via the fp32 magic-constant trick; the PE
                    # then accumulates -k into the same PSUM region so it
                    # holds r = ang - rint(ang) in [-0.5, 0.5].
                    kint = modpool.tile([109, 2 * NF], F16)
                    nc.vector.tensor_scalar(
                        out=kint[:], in0=ang[:],
                        scalar1=float(MAGIC), scalar2=float(MAGIC),
                        op0=mybir.AluOpType.add, op1=mybir.AluOpType.subtract,
                    )
                    nc.tensor.matmul(
                        ang[0:108, 0:NF], negi_t[0:108, 0:108],
                        kint[0:108, 0:NF], start=False, stop=True,
                    )
                    nc.tensor.matmul(
                        ang[0:109, NF:2 * NF], negi_t[:], kint[:, NF:2 * NF],
                        start=False, stop=True,
                    )

                    # enc = sin2pi(r)
                    enc = encpool.tile([109, 2 * NF], F16)
                    nc.scalar.activation(
                        out=enc[:], in_=ang[:], func=SIN2PI,
                        bias=0.0, scale=1.0,
                    )

                    esc = escpool.tile([128, 2 * NF], BF16)
                    nc.scalar.activation(
                        out=esc[:], in_=sc[:], func=EXP, bias=0.0, scale=1.0
                    )

                    g = gpool.tile([109, 2 * NF], F32)
                    nc.tensor.matmul(
                        g[0:108, 0:NF], lp_t[:, 0:108], esc[:, 0:NF],
                        start=True, stop=False,
                    )
                    nc.tensor.matmul(
                        g[0:108, 0:NF], lp_t[:, 108:216], esc[:, NF:2 * NF],
                        start=False, stop=True,
                    )
                    nc.tensor.matmul(
                        g[0:109, NF:2 * NF], lp_t[:, 216:325], esc[:, 0:NF],
                        start=True, stop=False,
                    )
                    nc.tensor.matmul(
                        g[0:109, NF:2 * NF], lp_t[:, 325:434], esc[:, NF:2 * NF],
                        start=False, stop=True,
                    )

                    p = ppool.tile([109, 2 * NF], BF16)
                    nc.vector.tensor_mul(p[:], g[:], enc[:])

                    # rgbd accumulator aliases into g's [0:4, 0:NF] region,
                    # which is dead once the pmult has read g.
                    nc.tensor.matmul(
                        g[0:4, 0:NF],
                        rp_t[0:108, 0:4], p[0:108, 0:NF],
                        start=True, stop=False,
                    )
                    nc.tensor.matmul(
                        g[0:4, 0:NF],
                        rp_t[0:109, 4:8], p[0:109, NF:2 * NF],
                        start=False, stop=True,
                    )

                    rgbs = rspool.tile([4, NF], F32)
                    nc.scalar.copy(out=rgbs[:], in_=g[0:4, 0:NF])
                    nc.sync.dma_start(out=o4_h[:, s:s + NF], in_=rgbs[:])

    _split_excess_waits(nc)
    return nc


_PROGRAM = None


def _get_program():
    global _PROGRAM
    if _PROGRAM is None:
        _PROGRAM = _build_program()
    return _PROGRAM


def kernel(X, linear_mappings, centroids, _want_trace=False):
    wea, web, c6, lp, rp, negi = _build_static_arrays(linear_mappings, centroids)
    x13 = _build_x13(X)

    nc = _get_program()
    in_maps = [
        {
            "x13": np.ascontiguousarray(x13[c]),
            "wea": wea, "web": web, "c6": c6, "lp": lp, "rp": rp,
            "negi": negi,
        }
        for c in range(NCORES)
    ]
    res = run_bass_kernel_spmd(
        nc, in_maps, core_ids=list(range(NCORES)), trace=_want_trace
    )

    out = np.empty((N, 3), dtype=np.float32)
    for c in range(NCORES):
        o4 = res.results[c]["o4"]                      # [4, NPAD] f32
        seg = o4[:, :NPC]
        out[c * NPC:(c + 1) * NPC, :] = (seg[0:3] / seg[3:4]).T
    if _want_trace:
        return out, res
    return out

